# revision 1
# baseline (speedup 1.0000x reference)
"""GRU-style GNN message-passing kernel for Trainium2 (8 NeuronCores, SPMD).

Reference computation (per node b, features 256, 8 neighbors):
    xr = x @ Wir.T + bir
    hr_n = hs_n @ Whr.T + bhr
    r_n = sigmoid(xr + hr_n)
    z = sigmoid(x @ Wiz.T + biz + h_sum @ Whz.T + bhz)
    s = sum_n r_n * hs_n
    n = tanh(x @ Win.T + bin + s @ Whn.T + bhn)
    out = (1 - z) * n + z * h_sum

Strategy: data-parallel over the node dim B=32768 across 8 cores (4096
rows each), batch-chunked 8x512 per core. Everything on-chip runs in
feature-major ("transposed") layout [256 features = 2 partition chunks
of 128, batch free dim], so every linear layer is a natural PE matmul.
Matmuls and the streamed tensors (x, hs) are bf16 (fp32 PSUM
accumulation); h_sum, the z/n gates and the final combine stay fp32 so
the dominant z*h_sum term keeps fp32-level accuracy. Engine placement
per chunk of 512 nodes:
  - PE: all 13 linear-layer matmuls, N=512 moving dim; the shared
    (xr + b_r) term is added into each neighbor's PSUM group via an
    identity matmul; z and n accumulate both their linear terms in PSUM.
  - ACT: sigmoid/tanh (bias per-partition = per-feature); xr+b_r add.
  - DVE: r*hs pair-products and the neighbor add tree, in place in the
    hs tile (bf16 2x mode, contiguous [128,1024+] runs).
  - GPSIMD: h_sum fp32->bf16 cast, final combine out = n + z*(h - n).
"""

import sys
import numpy as np
from contextlib import ExitStack

sys.path.insert(0, "/opt/trn_rl_repo")

import ml_dtypes
import concourse.bacc as bacc
import concourse.tile as tile
from concourse import mybir
from concourse.bass_utils import run_bass_kernel_spmd

F32 = mybir.dt.float32
BF16 = mybir.dt.bfloat16
BF_NP = ml_dtypes.bfloat16

N_NEIGH, B, IN, H = 8, 32768, 256, 256
M = 8                    # cores
BL = B // M              # rows per core (4096)
NCH = 8                  # batch chunks per core
CW = BL // NCH           # chunk width (512)

_cached = None  # compiled program, reused across kernel() calls


def _build():
    nc = bacc.Bacc("TRN2", target_bir_lowering=False, debug=False, num_devices=M)

    xT = nc.dram_tensor("xT", [IN, BL], BF16, kind="ExternalInput").ap()
    hT = nc.dram_tensor("hT", [H, BL], F32, kind="ExternalInput").ap()
    hsT = nc.dram_tensor("hsT", [N_NEIGH, H, BL], BF16, kind="ExternalInput").ap()
    wAP = {}
    for w in ("wir", "whr", "wiz", "whz", "win", "whn"):
        wAP[w] = nc.dram_tensor(w, [256, 256], BF16, kind="ExternalInput").ap()
    ident = nc.dram_tensor("ident", [128, 128], BF16, kind="ExternalInput").ap()
    # bias pack: col f*3+j holds feature-chunk f of (b_r, b_z, b_n)[j]
    biasp = nc.dram_tensor("biasp", [128, 6], F32, kind="ExternalInput").ap()
    outT = nc.dram_tensor("outT", [H, BL], F32, kind="ExternalOutput").ap()

    with tile.TileContext(nc) as tc, ExitStack() as ctx:
        const_pool = ctx.enter_context(tc.tile_pool(name="const", bufs=1))
        x_pool = ctx.enter_context(tc.tile_pool(name="x", bufs=2))
        h_pool = ctx.enter_context(tc.tile_pool(name="h", bufs=2))
        hb_pool = ctx.enter_context(tc.tile_pool(name="hb", bufs=2))
        hs_pool = ctx.enter_context(tc.tile_pool(name="hs", bufs=3))
        xr_pool = ctx.enter_context(tc.tile_pool(name="xr", bufs=2))
        z_pool = ctx.enter_context(tc.tile_pool(name="z", bufs=2))
        s_pool = ctx.enter_context(tc.tile_pool(name="s", bufs=2))
        r_pool = ctx.enter_context(tc.tile_pool(name="r", bufs=2))
        n_pool = ctx.enter_context(tc.tile_pool(name="n", bufs=2))
        d_pool = ctx.enter_context(tc.tile_pool(name="d", bufs=2))
        o_pool = ctx.enter_context(tc.tile_pool(name="o", bufs=2))
        pz_pool = ctx.enter_context(tc.tile_pool(name="pz", bufs=2, space="PSUM"))
        pr_pool = ctx.enter_context(tc.tile_pool(name="pr", bufs=2, space="PSUM"))
        pn_pool = ctx.enter_context(tc.tile_pool(name="pn", bufs=2, space="PSUM"))

        # --- constants ---
        wt = {}
        for w in ("wir", "whr", "wiz", "whz", "win", "whn"):
            wt[w] = []
            for k in range(2):
                t = const_pool.tile([128, 256], BF16, tag=f"{w}{k}", name=f"{w}{k}")
                nc.sync.dma_start(out=t[:, :], in_=wAP[w][k * 128:(k + 1) * 128, :])
                wt[w].append(t)
        id_t = const_pool.tile([128, 128], BF16, tag="ident", name="id_t")
        nc.sync.dma_start(out=id_t[:, :], in_=ident[:, :])
        bias_t = const_pool.tile([128, 6], F32, tag="biasp", name="bias_t")
        nc.sync.dma_start(out=bias_t[:, :], in_=biasp[:, :])

        def fcols(t, f):
            return t[:, f * 128:(f + 1) * 128]

        for c in range(NCH):
            sl = slice(c * CW, (c + 1) * CW)

            # x.T as one [128, 1024] bf16 tile, f-chunk halves (one 3D DMA)
            xt = x_pool.tile([128, 2 * CW], BF16, tag="x", name=f"x_{c}")
            nc.sync.dma_start(
                out=xt[:, :].rearrange("p (f b) -> p f b", f=2),
                in_=xT[:, sl].rearrange("(f p) b -> p f b", f=2))
            # h_sum.T fp32 [128, 1024] + bf16 cast for the Whz matmul
            ht = h_pool.tile([128, 2 * CW], F32, tag="h", name=f"h_{c}")
            nc.sync.dma_start(
                out=ht[:, :].rearrange("p (f b) -> p f b", f=2),
                in_=hT[:, sl].rearrange("(f p) b -> p f b", f=2))
            htb = hb_pool.tile([128, 2 * CW], BF16, tag="hb", name=f"hb_{c}")
            nc.gpsimd.tensor_copy(htb[:, :], ht[:, :])
            # hs.T as one [128, 8192] bf16 tile: (n, f, b) layout, one DMA
            # per neighbor
            hsc = hs_pool.tile([128, 2 * N_NEIGH * CW], BF16, tag="hs",
                               name=f"hs_{c}")
            for n in range(N_NEIGH):
                nc.sync.dma_start(
                    out=hsc[:, n * 2 * CW:(n + 1) * 2 * CW].rearrange(
                        "p (f b) -> p f b", f=2),
                    in_=hsT[n, :, sl].rearrange("(f p) b -> p f b", f=2))

            def hs_n(n):        # [128, 1024] both feature chunks of neighbor n
                return hsc[:, n * 2 * CW:(n + 1) * 2 * CW]

            def hs_slice(n, f):  # [128, 512] matmul operand
                return hsc[:, (n * 2 + f) * CW:(n * 2 + f + 1) * CW]

            # --- z gate: sigmoid(Wiz@x + Whz@h + b_z), PSUM-accumulated ---
            zt = z_pool.tile([128, 2 * CW], F32, tag="z", name=f"z_{c}")
            for f in range(2):
                pz = pz_pool.tile([128, CW], F32, tag="pz", name=f"pz{f}_{c}")
                nc.tensor.matmul(pz[:, :], fcols(wt["wiz"][0], f), xt[:, 0:CW],
                                 start=True, stop=False)
                nc.tensor.matmul(pz[:, :], fcols(wt["wiz"][1], f),
                                 xt[:, CW:2 * CW], start=False, stop=False)
                nc.tensor.matmul(pz[:, :], fcols(wt["whz"][0], f), htb[:, 0:CW],
                                 start=False, stop=False)
                nc.tensor.matmul(pz[:, :], fcols(wt["whz"][1], f),
                                 htb[:, CW:2 * CW], start=False, stop=True)
                nc.scalar.activation(zt[:, f * CW:(f + 1) * CW], pz[:, :],
                                     mybir.ActivationFunctionType.Sigmoid,
                                     bias=bias_t[:, f * 3 + 1:f * 3 + 2])

            # --- xr + b_r, bf16 in SBUF, added into each neighbor's PSUM
            #     group via the identity matmul ---
            xr = xr_pool.tile([128, 2 * CW], BF16, tag="xr", name=f"xr_{c}")
            for f in range(2):
                pxr = pn_pool.tile([128, CW], F32, tag="pn", name=f"pxr{f}_{c}")
                nc.tensor.matmul(pxr[:, :], fcols(wt["wir"][0], f), xt[:, 0:CW],
                                 start=True, stop=False)
                nc.tensor.matmul(pxr[:, :], fcols(wt["wir"][1], f),
                                 xt[:, CW:2 * CW], start=False, stop=True)
                nc.scalar.add(xr[:, f * CW:(f + 1) * CW], pxr[:, :],
                              bias_t[:, f * 3:f * 3 + 1])

            # --- neighbor loop: r = sigmoid(Whr@hs_n + xr + b_r) as bf16;
            #     prod/sum tree runs incrementally as pairs complete ---
            rc = r_pool.tile([128, 2 * N_NEIGH * CW], BF16, tag="r", name=f"r_{c}")
            sc = s_pool.tile([128, 2 * CW], BF16, tag="s", name=f"s_{c}")
            for n in range(N_NEIGH):
                pr = pr_pool.tile([128, 2 * CW], F32, tag="pr", name=f"pr{n}_{c}")
                for f in range(2):
                    ph = pr[:, f * CW:(f + 1) * CW]
                    nc.tensor.matmul(ph, fcols(wt["whr"][0], f), hs_slice(n, 0),
                                     start=True, stop=False)
                    nc.tensor.matmul(ph, fcols(wt["whr"][1], f), hs_slice(n, 1),
                                     start=False, stop=False)
                    nc.tensor.matmul(ph, id_t[:, :], xr[:, f * CW:(f + 1) * CW],
                                     start=False, stop=True)
                nc.scalar.activation(rc[:, n * 2 * CW:(n + 1) * 2 * CW], pr[:, :],
                                     mybir.ActivationFunctionType.Sigmoid)
                if n % 2 == 1:
                    # prod for the (n-1, n) pair: [128, 2048] in place in hsc
                    pair = hsc[:, (n - 1) * 2 * CW:(n + 1) * 2 * CW]
                    nc.vector.tensor_mul(
                        pair, rc[:, (n - 1) * 2 * CW:(n + 1) * 2 * CW], pair)
                    # tree level 1: hsc[n-1] += hsc[n]
                    with nc.allow_low_precision(reason="bf16 neighbor sums"):
                        nc.vector.tensor_add(hs_n(n - 1), hs_n(n - 1), hs_n(n))
            # tree levels 2 + 3 -> s [128, (f, 512)] bf16
            with nc.allow_low_precision(reason="bf16 neighbor sums"):
                nc.vector.tensor_add(hs_n(0), hs_n(0), hs_n(2))
                nc.vector.tensor_add(hs_n(4), hs_n(4), hs_n(6))
                nc.vector.tensor_add(sc[:, :], hs_n(0), hs_n(4))

            # --- n gate: tanh(Win@x + Whn@s + b_n), PSUM-accumulated ---
            nt = n_pool.tile([128, 2 * CW], F32, tag="n", name=f"n_{c}")
            for f in range(2):
                pn = pn_pool.tile([128, CW], F32, tag="pn", name=f"pn{f}_{c}")
                nc.tensor.matmul(pn[:, :], fcols(wt["win"][0], f), xt[:, 0:CW],
                                 start=True, stop=False)
                nc.tensor.matmul(pn[:, :], fcols(wt["win"][1], f),
                                 xt[:, CW:2 * CW], start=False, stop=False)
                nc.tensor.matmul(pn[:, :], fcols(wt["whn"][0], f), sc[:, 0:CW],
                                 start=False, stop=False)
                nc.tensor.matmul(pn[:, :], fcols(wt["whn"][1], f),
                                 sc[:, CW:2 * CW], start=False, stop=True)
                nc.scalar.activation(nt[:, f * CW:(f + 1) * CW], pn[:, :],
                                     mybir.ActivationFunctionType.Tanh,
                                     bias=bias_t[:, f * 3 + 2:f * 3 + 3])

            # --- out = n + z * (h - n) on GPSIMD, [128, 1024] fp32 ---
            dt_ = d_pool.tile([128, 2 * CW], F32, tag="d", name=f"d_{c}")
            nc.gpsimd.tensor_sub(dt_[:, :], ht[:, :], nt[:, :])
            nc.gpsimd.tensor_mul(dt_[:, :], zt[:, :], dt_[:, :])
            ot = o_pool.tile([128, 2 * CW], F32, tag="o", name=f"o_{c}")
            nc.gpsimd.tensor_add(ot[:, :], nt[:, :], dt_[:, :])
            for f in range(2):
                nc.sync.dma_start(out=outT[f * 128:(f + 1) * 128, sl],
                                  in_=ot[:, f * CW:(f + 1) * CW])

    nc.compile()
    return nc


def _prep_inputs(x, h_sum, hs, Wir, bir, Whr, bhr, Wiz, biz, Whz, bhz,
                 Win, bin_, Whn, bhn):
    """Shard + transpose to feature-major per-core input maps."""
    f32 = np.float32
    xT = np.asarray(x, f32).T.astype(BF_NP)                  # [256, B] bf16
    hT = np.ascontiguousarray(np.asarray(h_sum, f32).T)      # [256, B] f32
    hsT = np.asarray(hs, f32).transpose(0, 2, 1).astype(BF_NP)  # [8,256,B] bf16

    w = {
        "wir": np.ascontiguousarray(np.asarray(Wir, f32).T.astype(BF_NP)),
        "whr": np.ascontiguousarray(np.asarray(Whr, f32).T.astype(BF_NP)),
        "wiz": np.ascontiguousarray(np.asarray(Wiz, f32).T.astype(BF_NP)),
        "whz": np.ascontiguousarray(np.asarray(Whz, f32).T.astype(BF_NP)),
        "win": np.ascontiguousarray(np.asarray(Win, f32).T.astype(BF_NP)),
        "whn": np.ascontiguousarray(np.asarray(Whn, f32).T.astype(BF_NP)),
    }
    b_r = np.asarray(bir, f32) + np.asarray(bhr, f32)
    b_z = np.asarray(biz, f32) + np.asarray(bhz, f32)
    b_n = np.asarray(bin_, f32) + np.asarray(bhn, f32)
    biasp = np.empty((128, 6), f32)
    for f in range(2):
        biasp[:, f * 3 + 0] = b_r[f * 128:(f + 1) * 128]
        biasp[:, f * 3 + 1] = b_z[f * 128:(f + 1) * 128]
        biasp[:, f * 3 + 2] = b_n[f * 128:(f + 1) * 128]
    ident = np.eye(128, dtype=f32).astype(BF_NP)

    in_maps = []
    for c in range(M):
        sl = slice(c * BL, (c + 1) * BL)
        m = {
            "xT": np.ascontiguousarray(xT[:, sl]),
            "hT": np.ascontiguousarray(hT[:, sl]),
            "hsT": np.ascontiguousarray(hsT[:, :, sl]),
            "ident": ident,
            "biasp": biasp,
        }
        m.update(w)
        in_maps.append(m)
    return in_maps


def _run(inputs, trace=False, **trace_kwargs):
    global _cached
    if _cached is None:
        _cached = _build()
    nc = _cached
    in_maps = _prep_inputs(**inputs)
    res = run_bass_kernel_spmd(nc, in_maps, list(range(M)), trace=trace,
                               **trace_kwargs)
    out = np.empty((B, H), np.float32)
    for c in range(M):
        out[c * BL:(c + 1) * BL, :] = res.results[c]["outT"].T
    return out, res


def kernel(**inputs):
    return _run(inputs)[0]



# revision 3
# speedup vs baseline: 1.2593x; 1.2593x over previous
"""GRU-style GNN message-passing kernel for Trainium2 (8 NeuronCores, SPMD).

Reference computation (per node b, features 256, 8 neighbors):
    xr = x @ Wir.T + bir
    hr_n = hs_n @ Whr.T + bhr
    r_n = sigmoid(xr + hr_n)
    z = sigmoid(x @ Wiz.T + biz + h_sum @ Whz.T + bhz)
    s = sum_n r_n * hs_n
    n = tanh(x @ Win.T + bin + s @ Whn.T + bhn)
    out = (1 - z) * n + z * h_sum

Strategy: data-parallel over the node dim B=32768 across 8 cores (4096
rows each), batch-chunked 8x512 per core, feature-major on chip
(256 feats = 2 partition chunks of 128, batch in the free dim).

Key structure (vs. the naive chunk loop):
  - All HBM tensors are host-packed into the exact SBUF image per chunk
    (one big contiguous DMA per tensor per chunk, 2-16KB partition lines).
  - Software-pipelined emission: chunk c's n-gate + final combine are
    emitted after chunk c+1's head, so the PE instruction stream never
    waits on the DVE product/sum tree of the current chunk.
  - h_sum cast (f32->bf16) and the final combine run on DVE (fp32),
    GPSIMD is not used at all.
  - The shared (xr + b_r) term is injected into each neighbor-pair's
    PSUM group via an identity matmul; r-gate matmuls are ordered
    stationary-major so LDWEIGHTS is amortized.
  - PSUM: one pool of 4 x [128,1024] tiles (8 banks): xr, z, 8 r-units,
    n rotate through it.
  - PE warm-up dummy matmuls + ACT table warm-up overlap the initial
    DMA wait (HAM clock-gate, sigmoid table load).
"""

import sys
import numpy as np
from contextlib import ExitStack

sys.path.insert(0, "/opt/trn_rl_repo")

import ml_dtypes
import concourse.bacc as bacc
import concourse.tile as tile
from concourse import mybir
from concourse.bass_utils import run_bass_kernel_spmd

F32 = mybir.dt.float32
BF16 = mybir.dt.bfloat16
BF_NP = ml_dtypes.bfloat16

N_NEIGH, B, IN, H = 8, 32768, 256, 256
M = 8                    # cores
BL = B // M              # rows per core (4096)
NCH = 8                  # batch chunks per core
CW = BL // NCH           # chunk width (512)

SIG = mybir.ActivationFunctionType.Sigmoid
TANH = mybir.ActivationFunctionType.Tanh
IDENT = mybir.ActivationFunctionType.Identity

_cached = None  # compiled program, reused across kernel() calls


def _build():
    nc = bacc.Bacc("TRN2", target_bir_lowering=False, debug=False, num_devices=M)

    # Host-packed per-chunk DRAM images (see _prep_inputs for layouts).
    xD = nc.dram_tensor("xT", [NCH, 128, 2 * CW], BF16, kind="ExternalInput").ap()
    hD = nc.dram_tensor("hT", [NCH, 128, 2 * CW], F32, kind="ExternalInput").ap()
    hsD = nc.dram_tensor("hsT", [NCH, 128, 2 * N_NEIGH * CW], BF16,
                         kind="ExternalInput").ap()
    wAP = {}
    for w in ("wir", "whr", "wiz", "whz", "win", "whn"):
        wAP[w] = nc.dram_tensor(w, [256, 256], BF16, kind="ExternalInput").ap()
    ident = nc.dram_tensor("ident", [128, 128], BF16, kind="ExternalInput").ap()
    # bias pack: col f*3+j holds feature-chunk f of (b_r, b_z, b_n)[j]
    biasp = nc.dram_tensor("biasp", [128, 6], F32, kind="ExternalInput").ap()
    outD = nc.dram_tensor("outT", [NCH, 128, 2 * CW], F32,
                          kind="ExternalOutput").ap()

    with tile.TileContext(nc) as tc, ExitStack() as ctx:
        const_pool = ctx.enter_context(tc.tile_pool(name="const", bufs=1))
        x_pool = ctx.enter_context(tc.tile_pool(name="x", bufs=3))
        h_pool = ctx.enter_context(tc.tile_pool(name="h", bufs=3))
        hb_pool = ctx.enter_context(tc.tile_pool(name="hb", bufs=2))
        hs_pool = ctx.enter_context(tc.tile_pool(name="hs", bufs=3))
        xr_pool = ctx.enter_context(tc.tile_pool(name="xr", bufs=2))
        r_pool = ctx.enter_context(tc.tile_pool(name="r", bufs=2))
        s_pool = ctx.enter_context(tc.tile_pool(name="s", bufs=2))
        z_pool = ctx.enter_context(tc.tile_pool(name="z", bufs=2))
        n_pool = ctx.enter_context(tc.tile_pool(name="n", bufs=2))
        d_pool = ctx.enter_context(tc.tile_pool(name="d", bufs=2))
        o_pool = ctx.enter_context(tc.tile_pool(name="o", bufs=2))
        ps_pool = ctx.enter_context(tc.tile_pool(name="ps", bufs=4, space="PSUM"))

        # --- constants ---
        wt = {}
        for w in ("wir", "whr", "wiz", "whz", "win", "whn"):
            wt[w] = []
            for k in range(2):
                t = const_pool.tile([128, 256], BF16, tag=f"{w}{k}", name=f"{w}{k}")
                nc.sync.dma_start(out=t[:, :], in_=wAP[w][k * 128:(k + 1) * 128, :])
                wt[w].append(t)
        id_t = const_pool.tile([128, 128], BF16, tag="ident", name="id_t")
        nc.sync.dma_start(out=id_t[:, :], in_=ident[:, :])
        bias_t = const_pool.tile([128, 6], F32, tag="biasp", name="bias_t")
        nc.sync.dma_start(out=bias_t[:, :], in_=biasp[:, :])

        def wsl(w, k, f):      # stationary [128,128]: contract chunk k, out chunk f
            return wt[w][k][:, f * 128:(f + 1) * 128]

        # Warm-up: ACT table load (sigmoid+tanh share a set) and PE HAM
        # un-throttle, both overlapping the first chunk's input DMA.
        warm_act = const_pool.tile([128, 4], F32, tag="wact", name="warm_act")
        nc.scalar.activation(warm_act[:, 0:1], bias_t[:, 0:1], SIG)
        nc.scalar.activation(warm_act[:, 1:2], bias_t[:, 0:1], TANH)
        warm_ps = ps_pool.tile([128, 1024], F32, tag="ps", name="warm_ps")
        for i in range(24):
            nc.tensor.matmul(warm_ps[:, 0:128], id_t[:, :], id_t[:, :],
                             start=True, stop=True)

        # per-chunk state carried from head(c) to tail(c)
        st = {}

        def head(c):
            # -- DMAs: one contiguous transfer per tensor --
            xt = x_pool.tile([128, 2 * CW], BF16, tag="x", name=f"x_{c}")
            nc.sync.dma_start(out=xt[:, :], in_=xD[c])
            ht = h_pool.tile([128, 2 * CW], F32, tag="h", name=f"h_{c}")
            nc.sync.dma_start(out=ht[:, :], in_=hD[c])
            hsc = hs_pool.tile([128, 2 * N_NEIGH * CW], BF16, tag="hs",
                               name=f"hs_{c}")
            nc.sync.dma_start(out=hsc[:, :], in_=hsD[c])

            # h_sum bf16 cast for the Whz matmul (DVE copy, 2x_2P mode)
            htb = hb_pool.tile([128, 2 * CW], BF16, tag="hb", name=f"hb_{c}")
            nc.vector.tensor_copy(htb[:, :], ht[:, :])

            def xk(k):
                return xt[:, k * CW:(k + 1) * CW]

            def hs_sl(k, n, b0, b1):   # hs layout (k, n, b)
                base = (k * N_NEIGH + n) * CW
                return hsc[:, base + b0:base + b1]

            # -- xr = Wir@x + b_r  -> bf16 SBUF --
            xr = xr_pool.tile([128, 2 * CW], BF16, tag="xr", name=f"xr_{c}")
            pxr = ps_pool.tile([128, 2 * CW], F32, tag="ps", name=f"pxr_{c}")
            for f in range(2):
                for k in range(2):
                    nc.tensor.matmul(pxr[:, f * CW:(f + 1) * CW],
                                     wsl("wir", k, f), xk(k),
                                     start=(k == 0), stop=(k == 1))
            for f in range(2):
                nc.scalar.activation(xr[:, f * CW:(f + 1) * CW],
                                     pxr[:, f * CW:(f + 1) * CW], IDENT,
                                     bias=bias_t[:, f * 3:f * 3 + 1])

            # -- z = sigmoid(Wiz@x + Whz@h + b_z) -> f32 SBUF --
            zt = z_pool.tile([128, 2 * CW], F32, tag="z", name=f"z_{c}")
            pz = ps_pool.tile([128, 2 * CW], F32, tag="ps", name=f"pz_{c}")
            for f in range(2):
                nc.tensor.matmul(pz[:, f * CW:(f + 1) * CW], wsl("wiz", 0, f),
                                 xk(0), start=True, stop=False)
                nc.tensor.matmul(pz[:, f * CW:(f + 1) * CW], wsl("wiz", 1, f),
                                 xk(1), start=False, stop=False)
                nc.tensor.matmul(pz[:, f * CW:(f + 1) * CW], wsl("whz", 0, f),
                                 htb[:, 0:CW], start=False, stop=False)
                nc.tensor.matmul(pz[:, f * CW:(f + 1) * CW], wsl("whz", 1, f),
                                 htb[:, CW:2 * CW], start=False, stop=True)
            for f in range(2):
                nc.scalar.activation(zt[:, f * CW:(f + 1) * CW],
                                     pz[:, f * CW:(f + 1) * CW], SIG,
                                     bias=bias_t[:, f * 3 + 1:f * 3 + 2])

            # -- r units: for f, for neighbor pair j: [128, 2*CW] PSUM --
            #    pre_r = Whr@hs_n + (xr + b_r), stationary-major order
            rc = r_pool.tile([128, 2 * N_NEIGH * CW], BF16, tag="r",
                             name=f"r_{c}")
            sc = s_pool.tile([128, 2 * CW], BF16, tag="s", name=f"s_{c}")
            for f in range(2):
                for j in range(N_NEIGH // 2):
                    pr = ps_pool.tile([128, 2 * CW], F32, tag="ps",
                                      name=f"pr{f}{j}_{c}")
                    for k in range(2):
                        nc.tensor.matmul(pr[:, 0:CW], wsl("whr", k, f),
                                         hs_sl(k, 2 * j, 0, CW),
                                         start=(k == 0), stop=False)
                        nc.tensor.matmul(pr[:, CW:2 * CW], wsl("whr", k, f),
                                         hs_sl(k, 2 * j + 1, 0, CW),
                                         start=(k == 0), stop=False)
                    nc.tensor.matmul(pr[:, 0:CW], id_t[:, :],
                                     xr[:, f * CW:(f + 1) * CW],
                                     start=False, stop=True)
                    nc.tensor.matmul(pr[:, CW:2 * CW], id_t[:, :],
                                     xr[:, f * CW:(f + 1) * CW],
                                     start=False, stop=True)
                    base = f * N_NEIGH * CW + 2 * j * CW
                    nc.scalar.activation(rc[:, base:base + 2 * CW], pr[:, :],
                                         SIG)
                # -- DVE: products + neighbor sum tree for this f --
                #    (in place in rc: hsc stays intact for the other f's
                #    matmuls, which contract over both k halves)
                fb = f * N_NEIGH * CW
                half = N_NEIGH * CW // 2
                nc.vector.tensor_mul(rc[:, fb:fb + N_NEIGH * CW],
                                     rc[:, fb:fb + N_NEIGH * CW],
                                     hsc[:, fb:fb + N_NEIGH * CW])
                with nc.allow_low_precision(reason="bf16 neighbor sums"):
                    nc.vector.tensor_add(rc[:, fb:fb + half],
                                         rc[:, fb:fb + half],
                                         rc[:, fb + half:fb + 2 * half])
                    nc.vector.tensor_add(rc[:, fb:fb + half // 2],
                                         rc[:, fb:fb + half // 2],
                                         rc[:, fb + half // 2:fb + half])
                    nc.vector.tensor_add(sc[:, f * CW:(f + 1) * CW],
                                         rc[:, fb:fb + CW],
                                         rc[:, fb + CW:fb + 2 * CW])

            st[c] = (xt, ht, zt, sc)

        def tail(c):
            xt, ht, zt, sc = st.pop(c)

            def xk(k):
                return xt[:, k * CW:(k + 1) * CW]

            # -- n = tanh(Win@x + Whn@s + b_n) -> f32 SBUF --
            nt = n_pool.tile([128, 2 * CW], F32, tag="n", name=f"n_{c}")
            pn = ps_pool.tile([128, 2 * CW], F32, tag="ps", name=f"pn_{c}")
            for f in range(2):
                nc.tensor.matmul(pn[:, f * CW:(f + 1) * CW], wsl("win", 0, f),
                                 xk(0), start=True, stop=False)
                nc.tensor.matmul(pn[:, f * CW:(f + 1) * CW], wsl("win", 1, f),
                                 xk(1), start=False, stop=False)
                nc.tensor.matmul(pn[:, f * CW:(f + 1) * CW], wsl("whn", 0, f),
                                 sc[:, 0:CW], start=False, stop=False)
                nc.tensor.matmul(pn[:, f * CW:(f + 1) * CW], wsl("whn", 1, f),
                                 sc[:, CW:2 * CW], start=False, stop=True)
            for f in range(2):
                nc.scalar.activation(nt[:, f * CW:(f + 1) * CW],
                                     pn[:, f * CW:(f + 1) * CW], TANH,
                                     bias=bias_t[:, f * 3 + 2:f * 3 + 3])

            # -- out = n + z*(h - n), fp32 on DVE --
            dt_ = d_pool.tile([128, 2 * CW], F32, tag="d", name=f"d_{c}")
            nc.vector.tensor_sub(dt_[:, :], ht[:, :], nt[:, :])
            nc.vector.tensor_mul(dt_[:, :], zt[:, :], dt_[:, :])
            ot = o_pool.tile([128, 2 * CW], F32, tag="o", name=f"o_{c}")
            nc.vector.tensor_add(ot[:, :], nt[:, :], dt_[:, :])
            nc.sync.dma_start(out=outD[c], in_=ot[:, :])

        # software-pipelined emission
        for c in range(NCH):
            head(c)
            if c >= 1:
                tail(c - 1)
        tail(NCH - 1)

    nc.compile()
    return nc


def _prep_inputs(x, h_sum, hs, Wir, bir, Whr, bhr, Wiz, biz, Whz, bhz,
                 Win, bin_, Whn, bhn):
    """Shard + pack to per-core, per-chunk SBUF-image layouts."""
    f32 = np.float32

    w = {
        "wir": np.ascontiguousarray(np.asarray(Wir, f32).T.astype(BF_NP)),
        "whr": np.ascontiguousarray(np.asarray(Whr, f32).T.astype(BF_NP)),
        "wiz": np.ascontiguousarray(np.asarray(Wiz, f32).T.astype(BF_NP)),
        "whz": np.ascontiguousarray(np.asarray(Whz, f32).T.astype(BF_NP)),
        "win": np.ascontiguousarray(np.asarray(Win, f32).T.astype(BF_NP)),
        "whn": np.ascontiguousarray(np.asarray(Whn, f32).T.astype(BF_NP)),
    }
    b_r = np.asarray(bir, f32) + np.asarray(bhr, f32)
    b_z = np.asarray(biz, f32) + np.asarray(bhz, f32)
    b_n = np.asarray(bin_, f32) + np.asarray(bhn, f32)
    biasp = np.empty((128, 6), f32)
    for f in range(2):
        biasp[:, f * 3 + 0] = b_r[f * 128:(f + 1) * 128]
        biasp[:, f * 3 + 1] = b_z[f * 128:(f + 1) * 128]
        biasp[:, f * 3 + 2] = b_n[f * 128:(f + 1) * 128]
    ident = np.eye(128, dtype=f32).astype(BF_NP)

    # x: [B, 256] -> per core [NCH, 128, (k, b)] bf16
    xbf = np.asarray(x, f32).astype(BF_NP)
    x5 = xbf.reshape(M, NCH, CW, 2, 128)            # [core, c, b, k, p]
    x_pack = np.ascontiguousarray(x5.transpose(0, 1, 4, 3, 2)) \
        .reshape(M, NCH, 128, 2 * CW)
    # h_sum: same layout, f32
    hf = np.asarray(h_sum, f32)
    h5 = hf.reshape(M, NCH, CW, 2, 128)
    h_pack = np.ascontiguousarray(h5.transpose(0, 1, 4, 3, 2)) \
        .reshape(M, NCH, 128, 2 * CW)
    # hs: [8, B, 256] -> per core [NCH, 128, (k, n, b)] bf16
    hsbf = np.asarray(hs, f32).astype(BF_NP)
    hs6 = hsbf.reshape(N_NEIGH, M, NCH, CW, 2, 128)  # [n, core, c, b, k, p]
    hs_pack = np.ascontiguousarray(hs6.transpose(1, 2, 5, 4, 0, 3)) \
        .reshape(M, NCH, 128, 2 * N_NEIGH * CW)

    in_maps = []
    for core in range(M):
        m = {
            "xT": x_pack[core],
            "hT": h_pack[core],
            "hsT": hs_pack[core],
            "ident": ident,
            "biasp": biasp,
        }
        m.update(w)
        in_maps.append(m)
    return in_maps


def _run(inputs, trace=False, **trace_kwargs):
    global _cached
    if _cached is None:
        _cached = _build()
    nc = _cached
    in_maps = _prep_inputs(**inputs)
    res = run_bass_kernel_spmd(nc, in_maps, list(range(M)), trace=trace,
                               **trace_kwargs)
    out = np.empty((B, H), np.float32)
    for core in range(M):
        o = res.results[core]["outT"]          # [NCH, 128, (f, b)] f32
        o = o.reshape(NCH, 128, 2, CW).transpose(0, 3, 2, 1)  # [c, b, f, p]
        out[core * BL:(core + 1) * BL, :] = o.reshape(BL, H)
    return out, res


def kernel(**inputs):
    return _run(inputs)[0]
